# revision 8
# baseline (speedup 1.0000x reference)
"""Trainium2 Bass kernel for nn_DiscriminationLoss (segment_reduce).

v3 design (8 NeuronCores, pixel-sharded; full inputs in, full loss out):

  - Each core gets 1/8 of the 4M pixels.  The HOST packs pred into the
    PE stationary layout directly: fp16, pre-scaled by 2^14, in
    block-diagonal unit slabs [p, (u, b, c)] with c in 0..8 where c==8
    is a baked-in ones column (per-kernel counts).  This is numerically
    identical to v2's on-chip ScalarE cast (same scale, same RNE
    rounding) but removes the whole ACT cast stage and halves the pred
    DMA bytes: 9.4 MB/core fp16 vs 16 MB f32.  Each DMA group is a
    fully CONTIGUOUS HBM block (group-major host layout).
  - Labels ship as fp16 (0..32 exact) — no on-chip int cast at all.
  - One-hot lives in class-major layout [p, (j, t)] so every
    tensor_scalar(is_equal) writes a fully dense step-1 16-bit AP and
    hits the DVE 4x_2p mode (58 + FD/4 cycles).  The matmul moving
    operand reads it through a 2D AP (j: stride FC, b: 4 dense) — PE
    moving fetch is 1 col/cycle regardless of stride.
  - DVE alone cannot cover 32 classes (32 x 4096 x (58+256)cyc at
    0.96 GHz = 42 us > the 29 us DMA floor), so classes are split:
    DVE gets most, GpSimd (tensor_scalar on the Q7s) and ScalarE take
    the rest.  ScalarE builds exact one-hots in two ACTIVATEs:
    t = Square(lab - j); oh = Relu(1 - t)  (integer labels => exact
    0/1; Square and Relu are filler functions in every table set, so
    only one ACT_TABLE_LOAD is paid).
  - The PE runs TWO concurrent 64-column tiles (tile t = u%2), each
    streaming its own 128-col moving operand: aggregate 2 cols/cycle,
    ~27 us for the 131072 moving columns.
  - Warmup matmuls on a memset tile trip the PE HAM clock gate during
    the initial DMA fill; one row of the warm psum is copied into an
    ignored row of the output tile to keep them live.
  - Host sums per-core partials (the "psum" step of the sharding hint)
    and evaluates the tiny O(K^2) pairwise tail in f64.
"""

import sys
import functools

sys.path.insert(0, "/opt/trn_rl_repo")

import numpy as np

C = 8
K = 32
NCORES = 8
H = W = 2048
PTOT = H * W
PCORE = PTOT // NCORES  # 524288
SIGMA_DIS = 3.0
PRED_SCALE = float(2.0**14)

QB = 4            # pixel-blocks per matmul unit (block-diagonal trick)
NCH = C + 1       # 8 pred channels + ones column (counts)
NSTAT = NCH * QB  # stationary columns per unit = 36 (fits the 64-col tile)
NMOV = K * QB     # moving columns per unit = 128
FTOT = PCORE // 128  # 4096 block-cols
FG = 512          # block-cols per pred DMA group (1.18 MB contiguous)
FC = 1024         # block-cols per one-hot chunk
LG = 1024         # block-cols per label DMA (256 KB contiguous)
NGROUPS = FTOT // FG
NCHUNKS = FTOT // FC
NUNITS = FTOT // QB
SLAB_BUFS = 6
WARM_MMS = 96

# one-hot class split across engines (class j handles label j+1)
DVE_K = 23
GPS_K = 6
ACT_K = K - DVE_K - GPS_K


def build_nc():
    import concourse.bacc as bacc
    import concourse.tile as tile
    import concourse.mybir as mybir
    from contextlib import ExitStack

    f32 = mybir.dt.float32
    fp16 = mybir.dt.float16

    nc = bacc.Bacc(
        "TRN2", target_bir_lowering=False, debug=False, num_devices=NCORES
    )
    # Group-major pred slabs: row block g*128..g*128+127 is DMA group g,
    # a single contiguous 1.18 MB HBM region.
    pred_ext = nc.dram_tensor(
        "pred", [NGROUPS * 128, FG // QB * NSTAT], fp16, kind="ExternalInput"
    )
    lab_ext = nc.dram_tensor(
        "labels", [(FTOT // LG) * 128, LG], fp16, kind="ExternalInput"
    )
    out_ext = nc.dram_tensor(
        "out_s", [128, NMOV + 8], f32, kind="ExternalOutput"
    )

    with tile.TileContext(nc) as tc, ExitStack() as ctx:
        const_pool = ctx.enter_context(tc.tile_pool(name="const", bufs=1))
        slab_pool = ctx.enter_context(
            tc.tile_pool(name="slab", bufs=SLAB_BUFS)
        )
        oh_pool = ctx.enter_context(tc.tile_pool(name="oh", bufs=2))
        tmp_pool = ctx.enter_context(tc.tile_pool(name="atmp", bufs=2))
        psum_pool = ctx.enter_context(tc.tile_pool(name="psum", bufs=1, space="PSUM"))

        labf = const_pool.tile([128, FTOT], fp16)
        outt = const_pool.tile([128, NMOV + 8], f32)
        warm_t = const_pool.tile([128, 128], mybir.dt.bfloat16)
        # per-column activation bias constants: -(j+1) for the ACT
        # classes, then +1.0 for the Relu(1 - x) step
        bias_t = const_pool.tile([128, ACT_K + 1], f32)
        for i in range(ACT_K):
            nc.vector.memset(
                bias_t[:, i : i + 1], -float(DVE_K + GPS_K + i + 1)
            )
        nc.vector.memset(bias_t[:, ACT_K : ACT_K + 1], 1.0)

        psum_s = psum_pool.tile([128, NMOV], f32)
        warm_ps = psum_pool.tile([128, 128], f32)

        # ---- DMA streams first: labels (ACT ring) + head pred groups
        # (SP ring) so the SDMA engines have work from t~=1us.
        for i in range(FTOT // LG):
            nc.scalar.dma_start(
                labf[:, i * LG : (i + 1) * LG],
                lab_ext[i * 128 : (i + 1) * 128, :],
            )

        slabs = []

        def emit_pred_dma(g):
            st = slab_pool.tile([128, FG // QB * NSTAT], fp16, tag="slab")
            nc.sync.dma_start(st, pred_ext[g * 128 : (g + 1) * 128, :])
            slabs.append(st)

        next_dma = 0
        while next_dma < 2 * (FC // FG):  # chunk-0 + chunk-1 groups
            emit_pred_dma(next_dma)
            next_dma += 1

        # ---- PE warmup on memset data during the DMA fill.
        nc.gpsimd.memset(warm_t[:], 1.0)
        nc.vector.memset(outt[:], 0.0)
        for w in range(WARM_MMS):
            nc.tensor.matmul(
                warm_ps[:64, :],
                warm_t[:, :64],
                warm_t[:, :128],
                start=(w == 0),
                stop=(w == WARM_MMS - 1),
            )

        # ---- main loop over one-hot chunks -------------------------------
        u = 0
        for ci in range(NCHUNKS):
            coff = ci * FC
            # pred DMA lookahead: everything the NEXT chunk needs.
            while next_dma < NGROUPS and next_dma * FG < coff + 2 * FC:
                emit_pred_dma(next_dma)
                next_dma += 1

            oh = oh_pool.tile([128, K * FC], fp16, tag="oh")
            oh2 = oh.rearrange("p (j t) -> p j t", j=K)
            lab_sl = labf[:, coff : coff + FC]
            for j in range(DVE_K):
                nc.vector.tensor_scalar(
                    oh2[:, j, :],
                    lab_sl,
                    float(j + 1),
                    None,
                    mybir.AluOpType.is_equal,
                )
            for j in range(DVE_K, DVE_K + GPS_K):
                nc.gpsimd.tensor_scalar(
                    oh2[:, j, :],
                    lab_sl,
                    float(j + 1),
                    None,
                    mybir.AluOpType.is_equal,
                )
            for j in range(DVE_K + GPS_K, K):
                at = tmp_pool.tile([128, FC], fp16, tag="atmp")
                nc.scalar.activation(
                    at[:],
                    lab_sl,
                    mybir.ActivationFunctionType.Square,
                    bias=bias_t[:, j - DVE_K - GPS_K : j - DVE_K - GPS_K + 1],
                )
                nc.scalar.activation(
                    oh2[:, j, :],
                    at[:],
                    mybir.ActivationFunctionType.Relu,
                    scale=-1.0,
                    bias=bias_t[:, ACT_K : ACT_K + 1],
                )

            for uc in range(FC // QB):
                g = u // (FG // QB)
                ug = u % (FG // QB)
                t = u % 2
                nc.tensor.matmul(
                    psum_s[64 * t : 64 * t + NSTAT, :],
                    slabs[g][:, ug * NSTAT : (ug + 1) * NSTAT],
                    oh2[:, :, uc * QB : (uc + 1) * QB],
                    start=(u < 2),
                    stop=(u >= NUNITS - 2),
                    tile_position=(0, 64 * t),
                    skip_group_check=True,
                )
                u += 1

        # ---- output ------------------------------------------------------
        nc.vector.tensor_copy(outt[:NSTAT, :NMOV], psum_s[:NSTAT, :])
        nc.vector.tensor_copy(
            outt[64 : 64 + NSTAT, :NMOV], psum_s[64 : 64 + NSTAT, :]
        )
        # keep the warm matmuls live (scratch cols, 32-aligned psum base)
        nc.vector.tensor_copy(outt[32:33, NMOV:], warm_ps[32:33, :8])
        nc.sync.dma_start(out_ext[:], outt[:])
    nc.compile()
    return nc


@functools.lru_cache(maxsize=1)
def _get_program():
    return build_nc()


def pack_core(pred_core, labels_core):
    """Host-side packing into the kernel's DMA layouts.

    pred -> fp16 * 2^14 in group-major stationary slabs
    [g, p, (u, b, c)] with the ones column baked in at c == 8;
    labels -> fp16 in group-major [gl, p, t] blocks.
    Pixel (p, t) = core_linear[p * FTOT + t].
    """
    ph = (
        np.asarray(pred_core, dtype=np.float32).reshape(C, 128, FTOT)
        * np.float32(PRED_SCALE)
    ).astype(np.float16)
    arr = np.empty((128, FTOT, NCH), dtype=np.float16)
    arr[:, :, :C] = ph.transpose(1, 2, 0)
    arr[:, :, C] = np.float16(1.0)
    pred_r = np.ascontiguousarray(
        arr.reshape(128, NGROUPS, FG * NCH).transpose(1, 0, 2)
    ).reshape(NGROUPS * 128, FG // QB * NSTAT)
    labf = labels_core.reshape(128, FTOT).astype(np.float16)
    lab_r = np.ascontiguousarray(
        labf.reshape(128, FTOT // LG, LG).transpose(1, 0, 2)
    ).reshape((FTOT // LG) * 128, LG)
    return pred_r, lab_r


def make_in_maps(pred_flat, labels_flat, pcore=PCORE, ncores=NCORES):
    in_maps = []
    for i in range(ncores):
        sl = slice(i * pcore, (i + 1) * pcore)
        pred_r, lab_r = pack_core(pred_flat[:, sl], labels_flat[sl])
        in_maps.append({"pred": pred_r, "labels": lab_r})
    return in_maps


def extract_SN(res_core):
    """From one core's outputs: S_scaled [C, K] and N [K]."""
    ps = res_core["out_s"].astype(np.float64)[:, :NMOV]  # [128, NMOV]
    S = np.zeros((C, K))
    N = np.zeros(K)
    for t in range(2):
        r = ps[64 * t : 64 * t + NSTAT, :].reshape(QB, NCH, K, QB)
        d = r[np.arange(QB), :, :, np.arange(QB)].sum(axis=0)  # [NCH, K]
        S += d[:C, :]
        N += d[C, :]
    return S, N


def finish_host(results, num_kernel):
    S = np.zeros((C, K))
    N = np.zeros(K)
    for r in results:
        Si, Ni = extract_SN(r)
        S += Si
        N += Ni
    S /= PRED_SCALE
    A = N * np.sum(S * S, axis=0)  # [K]
    kk = int(num_kernel)
    A = A[:kk]
    pair = A[:, None] + A[None, :]
    Dm = np.maximum(SIGMA_DIS - np.sqrt(pair), 0.0)
    term = np.log(Dm * Dm + 1.0)
    L = float(np.sum(np.triu(term, k=1)))
    L *= (kk - 1) / kk
    return np.float32(L)


_last_results = None


def kernel(pred_similarities, regions_mask, kernel_labels, num_kernel, **kw):
    global _last_results
    from concourse.bass_utils import run_bass_kernel_spmd

    pred_flat = np.asarray(pred_similarities, dtype=np.float32).reshape(C, PTOT)
    labels_flat = np.asarray(kernel_labels, dtype=np.int32).reshape(PTOT)

    nc = _get_program()
    in_maps = make_in_maps(pred_flat, labels_flat)
    res = run_bass_kernel_spmd(nc, in_maps, list(range(NCORES)))
    _last_results = res
    return finish_host(
        [res.results[i] for i in range(NCORES)], num_kernel
    )


# revision 9
# speedup vs baseline: 1.0016x; 1.0016x over previous
"""Trainium2 Bass kernel for nn_DiscriminationLoss (segment_reduce).

v3 design (8 NeuronCores, pixel-sharded; full inputs in, full loss out):

  - Each core gets 1/8 of the 4M pixels.  The HOST packs pred into the
    PE stationary layout directly: fp16, pre-scaled by 2^14, in
    block-diagonal unit slabs [p, (u, b, c)] with c in 0..8 where c==8
    is a baked-in ones column (per-kernel counts).  This is numerically
    identical to v2's on-chip ScalarE cast (same scale, same RNE
    rounding) but removes the whole ACT cast stage and halves the pred
    DMA bytes: 9.4 MB/core fp16 vs 16 MB f32.  Each DMA group is a
    fully CONTIGUOUS HBM block (group-major host layout).
  - Labels ship as bf16 (0..32 exact) — no on-chip int cast at all
    (fp16 tensor_scalar inputs take a ~13x slow path on DVE/GpSimd;
    bf16-in -> fp16-out is the fast combination).
  - One-hot lives in class-major layout [p, (j, t)] so every
    tensor_scalar(is_equal) writes a fully dense step-1 16-bit AP and
    hits the DVE 4x_2p mode (58 + FD/4 cycles).  The matmul moving
    operand reads it through a 2D AP (j: stride FC, b: 4 dense) — PE
    moving fetch is 1 col/cycle regardless of stride.
  - DVE alone cannot cover 32 classes (32 x 4096 x (58+256)cyc at
    0.96 GHz = 42 us > the 29 us DMA floor), so classes are split:
    DVE gets most, GpSimd (tensor_scalar on the Q7s) and ScalarE take
    the rest.  ScalarE builds exact one-hots in two ACTIVATEs:
    t = Square(lab - j); oh = Relu(1 - t)  (integer labels => exact
    0/1; Square and Relu are filler functions in every table set, so
    only one ACT_TABLE_LOAD is paid).
  - The PE runs TWO concurrent 64-column tiles (tile t = u%2), each
    streaming its own 128-col moving operand: aggregate 2 cols/cycle,
    ~27 us for the 131072 moving columns.
  - Warmup matmuls on a memset tile trip the PE HAM clock gate during
    the initial DMA fill; one row of the warm psum is copied into an
    ignored row of the output tile to keep them live.
  - Host sums per-core partials (the "psum" step of the sharding hint)
    and evaluates the tiny O(K^2) pairwise tail in f64.
"""

import sys
import functools

sys.path.insert(0, "/opt/trn_rl_repo")

import numpy as np

C = 8
K = 32
NCORES = 8
H = W = 2048
PTOT = H * W
PCORE = PTOT // NCORES  # 524288
SIGMA_DIS = 3.0
PRED_SCALE = float(2.0**14)

QB = 4            # pixel-blocks per matmul unit (block-diagonal trick)
NCH = C + 1       # 8 pred channels + ones column (counts)
NSTAT = NCH * QB  # stationary columns per unit = 36 (fits the 64-col tile)
NMOV = K * QB     # moving columns per unit = 128
FTOT = PCORE // 128  # 4096 block-cols
FG = 512          # block-cols per pred DMA group (1.18 MB contiguous)
FC = 1024         # block-cols per one-hot chunk
LG = 1024         # block-cols per label DMA (256 KB contiguous)
NGROUPS = FTOT // FG
NCHUNKS = FTOT // FC
NUNITS = FTOT // QB
SLAB_BUFS = 6
WARM_MMS = 96

# one-hot class split across engines (class j handles label j+1)
DVE_K = 23
GPS_K = 6
ACT_K = K - DVE_K - GPS_K


def build_nc():
    import concourse.bacc as bacc
    import concourse.tile as tile
    import concourse.mybir as mybir
    from contextlib import ExitStack

    f32 = mybir.dt.float32
    fp16 = mybir.dt.float16

    nc = bacc.Bacc(
        "TRN2", target_bir_lowering=False, debug=False, num_devices=NCORES
    )
    # Group-major pred slabs: row block g*128..g*128+127 is DMA group g,
    # a single contiguous 1.18 MB HBM region.
    pred_ext = nc.dram_tensor(
        "pred", [NGROUPS * 128, FG // QB * NSTAT], fp16, kind="ExternalInput"
    )
    bf16 = mybir.dt.bfloat16
    lab_ext = nc.dram_tensor(
        "labels", [(FTOT // LG) * 128, LG], bf16, kind="ExternalInput"
    )
    out_ext = nc.dram_tensor(
        "out_s", [128, NMOV + 8], f32, kind="ExternalOutput"
    )

    with tile.TileContext(nc) as tc, ExitStack() as ctx:
        const_pool = ctx.enter_context(tc.tile_pool(name="const", bufs=1))
        slab_pool = ctx.enter_context(
            tc.tile_pool(name="slab", bufs=SLAB_BUFS)
        )
        oh_pool = ctx.enter_context(tc.tile_pool(name="oh", bufs=2))
        tmp_pool = ctx.enter_context(tc.tile_pool(name="atmp", bufs=2))
        psum_pool = ctx.enter_context(tc.tile_pool(name="psum", bufs=1, space="PSUM"))

        labf = const_pool.tile([128, FTOT], bf16)
        outt = const_pool.tile([128, NMOV + 8], f32)
        warm_t = const_pool.tile([128, 128], mybir.dt.bfloat16)
        # per-column activation bias constants: -(j+1) for the ACT
        # classes, then +1.0 for the Relu(1 - x) step
        bias_t = const_pool.tile([128, ACT_K + 1], f32)
        for i in range(ACT_K):
            nc.vector.memset(
                bias_t[:, i : i + 1], -float(DVE_K + GPS_K + i + 1)
            )
        nc.vector.memset(bias_t[:, ACT_K : ACT_K + 1], 1.0)

        psum_s = psum_pool.tile([128, NMOV], f32)
        warm_ps = psum_pool.tile([128, 128], f32)

        # ---- DMA streams first: labels (ACT ring) + head pred groups
        # (SP ring) so the SDMA engines have work from t~=1us.
        for i in range(FTOT // LG):
            nc.scalar.dma_start(
                labf[:, i * LG : (i + 1) * LG],
                lab_ext[i * 128 : (i + 1) * 128, :],
            )

        slabs = []

        def emit_pred_dma(g):
            st = slab_pool.tile([128, FG // QB * NSTAT], fp16, tag="slab")
            nc.sync.dma_start(st, pred_ext[g * 128 : (g + 1) * 128, :])
            slabs.append(st)

        next_dma = 0
        while next_dma < 2 * (FC // FG):  # chunk-0 + chunk-1 groups
            emit_pred_dma(next_dma)
            next_dma += 1

        # ---- PE warmup on memset data during the DMA fill.
        nc.gpsimd.memset(warm_t[:], 1.0)
        nc.vector.memset(outt[:], 0.0)
        for w in range(WARM_MMS):
            nc.tensor.matmul(
                warm_ps[:64, :],
                warm_t[:, :64],
                warm_t[:, :128],
                start=(w == 0),
                stop=(w == WARM_MMS - 1),
            )

        # ---- main loop over one-hot chunks -------------------------------
        u = 0
        for ci in range(NCHUNKS):
            coff = ci * FC
            # pred DMA lookahead: everything the NEXT chunk needs.
            while next_dma < NGROUPS and next_dma * FG < coff + 2 * FC:
                emit_pred_dma(next_dma)
                next_dma += 1

            oh = oh_pool.tile([128, K * FC], fp16, tag="oh")
            oh2 = oh.rearrange("p (j t) -> p j t", j=K)
            lab_sl = labf[:, coff : coff + FC]
            for j in range(DVE_K):
                nc.vector.tensor_scalar(
                    oh2[:, j, :],
                    lab_sl,
                    float(j + 1),
                    None,
                    mybir.AluOpType.is_equal,
                )
            for j in range(DVE_K, DVE_K + GPS_K):
                nc.gpsimd.tensor_scalar(
                    oh2[:, j, :],
                    lab_sl,
                    float(j + 1),
                    None,
                    mybir.AluOpType.is_equal,
                )
            for j in range(DVE_K + GPS_K, K):
                at = tmp_pool.tile([128, FC], fp16, tag="atmp")
                nc.scalar.activation(
                    at[:],
                    lab_sl,
                    mybir.ActivationFunctionType.Square,
                    bias=bias_t[:, j - DVE_K - GPS_K : j - DVE_K - GPS_K + 1],
                )
                nc.scalar.activation(
                    oh2[:, j, :],
                    at[:],
                    mybir.ActivationFunctionType.Relu,
                    scale=-1.0,
                    bias=bias_t[:, ACT_K : ACT_K + 1],
                )

            for uc in range(FC // QB):
                g = u // (FG // QB)
                ug = u % (FG // QB)
                t = u % 2
                nc.tensor.matmul(
                    psum_s[64 * t : 64 * t + NSTAT, :],
                    slabs[g][:, ug * NSTAT : (ug + 1) * NSTAT],
                    oh2[:, :, uc * QB : (uc + 1) * QB],
                    start=(u < 2),
                    stop=(u >= NUNITS - 2),
                    tile_position=(0, 64 * t),
                    skip_group_check=True,
                )
                u += 1

        # ---- output ------------------------------------------------------
        nc.vector.tensor_copy(outt[:NSTAT, :NMOV], psum_s[:NSTAT, :])
        nc.vector.tensor_copy(
            outt[64 : 64 + NSTAT, :NMOV], psum_s[64 : 64 + NSTAT, :]
        )
        # keep the warm matmuls live (scratch cols, 32-aligned psum base)
        nc.vector.tensor_copy(outt[32:33, NMOV:], warm_ps[32:33, :8])
        nc.sync.dma_start(out_ext[:], outt[:])
    nc.compile()
    return nc


@functools.lru_cache(maxsize=1)
def _get_program():
    return build_nc()


def pack_core(pred_core, labels_core):
    """Host-side packing into the kernel's DMA layouts.

    pred -> fp16 * 2^14 in group-major stationary slabs
    [g, p, (u, b, c)] with the ones column baked in at c == 8;
    labels -> fp16 in group-major [gl, p, t] blocks.
    Pixel (p, t) = core_linear[p * FTOT + t].
    """
    ph = (
        np.asarray(pred_core, dtype=np.float32).reshape(C, 128, FTOT)
        * np.float32(PRED_SCALE)
    ).astype(np.float16)
    arr = np.empty((128, FTOT, NCH), dtype=np.float16)
    arr[:, :, :C] = ph.transpose(1, 2, 0)
    arr[:, :, C] = np.float16(1.0)
    pred_r = np.ascontiguousarray(
        arr.reshape(128, NGROUPS, FG * NCH).transpose(1, 0, 2)
    ).reshape(NGROUPS * 128, FG // QB * NSTAT)
    import ml_dtypes

    labf = labels_core.reshape(128, FTOT).astype(ml_dtypes.bfloat16)
    lab_r = np.ascontiguousarray(
        labf.reshape(128, FTOT // LG, LG).transpose(1, 0, 2)
    ).reshape((FTOT // LG) * 128, LG)
    return pred_r, lab_r


def make_in_maps(pred_flat, labels_flat, pcore=PCORE, ncores=NCORES):
    in_maps = []
    for i in range(ncores):
        sl = slice(i * pcore, (i + 1) * pcore)
        pred_r, lab_r = pack_core(pred_flat[:, sl], labels_flat[sl])
        in_maps.append({"pred": pred_r, "labels": lab_r})
    return in_maps


def extract_SN(res_core):
    """From one core's outputs: S_scaled [C, K] and N [K]."""
    ps = res_core["out_s"].astype(np.float64)[:, :NMOV]  # [128, NMOV]
    S = np.zeros((C, K))
    N = np.zeros(K)
    for t in range(2):
        r = ps[64 * t : 64 * t + NSTAT, :].reshape(QB, NCH, K, QB)
        d = r[np.arange(QB), :, :, np.arange(QB)].sum(axis=0)  # [NCH, K]
        S += d[:C, :]
        N += d[C, :]
    return S, N


def finish_host(results, num_kernel):
    S = np.zeros((C, K))
    N = np.zeros(K)
    for r in results:
        Si, Ni = extract_SN(r)
        S += Si
        N += Ni
    S /= PRED_SCALE
    A = N * np.sum(S * S, axis=0)  # [K]
    kk = int(num_kernel)
    A = A[:kk]
    pair = A[:, None] + A[None, :]
    Dm = np.maximum(SIGMA_DIS - np.sqrt(pair), 0.0)
    term = np.log(Dm * Dm + 1.0)
    L = float(np.sum(np.triu(term, k=1)))
    L *= (kk - 1) / kk
    return np.float32(L)


_last_results = None


def kernel(pred_similarities, regions_mask, kernel_labels, num_kernel, **kw):
    global _last_results
    from concourse.bass_utils import run_bass_kernel_spmd

    pred_flat = np.asarray(pred_similarities, dtype=np.float32).reshape(C, PTOT)
    labels_flat = np.asarray(kernel_labels, dtype=np.int32).reshape(PTOT)

    nc = _get_program()
    in_maps = make_in_maps(pred_flat, labels_flat)
    res = run_bass_kernel_spmd(nc, in_maps, list(range(NCORES)))
    _last_results = res
    return finish_host(
        [res.results[i] for i in range(NCORES)], num_kernel
    )


# revision 10
# speedup vs baseline: 6.3122x; 6.3024x over previous
"""Trainium2 Bass kernel for nn_DiscriminationLoss (segment_reduce).

v3 design (8 NeuronCores, pixel-sharded; full inputs in, full loss out):

  - Each core gets 1/8 of the 4M pixels.  The HOST packs pred into the
    PE stationary layout directly: fp16, pre-scaled by 2^14, in
    block-diagonal unit slabs [p, (u, b, c)] with c in 0..8 where c==8
    is a baked-in ones column (per-kernel counts).  This is numerically
    identical to v2's on-chip ScalarE cast (same scale, same RNE
    rounding) but removes the whole ACT cast stage and halves the pred
    DMA bytes: 9.4 MB/core fp16 vs 16 MB f32.  Each DMA group is a
    fully CONTIGUOUS HBM block (group-major host layout).
  - Labels ship as bf16 (0..32 exact) — no on-chip int cast at all
    (fp16 tensor_scalar inputs take a ~13x slow path on DVE/GpSimd;
    bf16-in -> fp16-out is the fast combination).
  - One-hot lives in class-major layout [p, (j, t)] so every
    tensor_scalar(is_equal) writes a fully dense step-1 16-bit AP and
    hits the DVE 4x_2p mode (58 + FD/4 cycles).  The matmul moving
    operand reads it through a 2D AP (j: stride FC, b: 4 dense) — PE
    moving fetch is 1 col/cycle regardless of stride.
  - DVE alone cannot cover 32 classes (32 x 4096 x (58+256)cyc at
    0.96 GHz = 42 us > the 29 us DMA floor), so classes are split:
    DVE gets most, GpSimd (tensor_scalar on the Q7s) and ScalarE take
    the rest.  ScalarE builds exact one-hots in two ACTIVATEs:
    t = Square(lab - j); oh = Relu(1 - t)  (integer labels => exact
    0/1; Square and Relu are filler functions in every table set, so
    only one ACT_TABLE_LOAD is paid).
  - The PE runs TWO concurrent 64-column tiles (tile t = u%2), each
    streaming its own 128-col moving operand: aggregate 2 cols/cycle,
    ~27 us for the 131072 moving columns.
  - Warmup matmuls on a memset tile trip the PE HAM clock gate during
    the initial DMA fill; one row of the warm psum is copied into an
    ignored row of the output tile to keep them live.
  - Host sums per-core partials (the "psum" step of the sharding hint)
    and evaluates the tiny O(K^2) pairwise tail in f64.
"""

import sys
import functools

sys.path.insert(0, "/opt/trn_rl_repo")

import numpy as np

C = 8
K = 32
NCORES = 8
H = W = 2048
PTOT = H * W
PCORE = PTOT // NCORES  # 524288
SIGMA_DIS = 3.0
PRED_SCALE = float(2.0**14)

QB = 4            # pixel-blocks per matmul unit (block-diagonal trick)
NCH = C + 1       # 8 pred channels + ones column (counts)
NSTAT = NCH * QB  # stationary columns per unit = 36 (fits the 64-col tile)
NMOV = K * QB     # moving columns per unit = 128
FTOT = PCORE // 128  # 4096 block-cols
FG = 512          # block-cols per pred DMA group (1.18 MB contiguous)
FC = 1024         # block-cols per one-hot chunk
LG = 1024         # block-cols per label DMA (256 KB contiguous)
NGROUPS = FTOT // FG
NCHUNKS = FTOT // FC
NUNITS = FTOT // QB
SLAB_BUFS = 6
WARM_MMS = 96

# one-hot class split across engines (class j handles label j+1)
DVE_K = 27
GPS_K = 0  # GpSimd tensor ops serialize the DVE (SBUF port mutex) - banned
ACT_K = K - DVE_K - GPS_K


def build_nc():
    import concourse.bacc as bacc
    import concourse.tile as tile
    import concourse.mybir as mybir
    from contextlib import ExitStack

    f32 = mybir.dt.float32
    fp16 = mybir.dt.float16

    nc = bacc.Bacc(
        "TRN2", target_bir_lowering=False, debug=False, num_devices=NCORES
    )
    # Group-major pred slabs: row block g*128..g*128+127 is DMA group g,
    # a single contiguous 1.18 MB HBM region.
    pred_ext = nc.dram_tensor(
        "pred", [NGROUPS * 128, FG // QB * NSTAT], fp16, kind="ExternalInput"
    )
    bf16 = mybir.dt.bfloat16
    lab_ext = nc.dram_tensor(
        "labels", [(FTOT // LG) * 128, LG], bf16, kind="ExternalInput"
    )
    out_ext = nc.dram_tensor(
        "out_s", [128, NMOV + 8], f32, kind="ExternalOutput"
    )

    with tile.TileContext(nc) as tc, ExitStack() as ctx:
        const_pool = ctx.enter_context(tc.tile_pool(name="const", bufs=1))
        slab_pool = ctx.enter_context(
            tc.tile_pool(name="slab", bufs=SLAB_BUFS)
        )
        oh_pool = ctx.enter_context(tc.tile_pool(name="oh", bufs=2))
        tmp_pool = ctx.enter_context(tc.tile_pool(name="atmp", bufs=2))
        psum_pool = ctx.enter_context(tc.tile_pool(name="psum", bufs=1, space="PSUM"))

        labf = const_pool.tile([128, FTOT], bf16)
        outt = const_pool.tile([128, NMOV + 8], f32)
        warm_t = const_pool.tile([128, 128], mybir.dt.bfloat16)
        # per-column activation bias constants: -(j+1) for the ACT
        # classes, then +1.0 for the Relu(1 - x) step
        bias_t = const_pool.tile([128, ACT_K + 1], f32)
        for i in range(ACT_K):
            nc.vector.memset(
                bias_t[:, i : i + 1], -float(DVE_K + GPS_K + i + 1)
            )
        nc.vector.memset(bias_t[:, ACT_K : ACT_K + 1], 1.0)

        psum_s = psum_pool.tile([128, NMOV], f32)
        warm_ps = psum_pool.tile([128, 128], f32)

        # ---- DMA streams first: labels (ACT ring) + head pred groups
        # (SP ring) so the SDMA engines have work from t~=1us.
        for i in range(FTOT // LG):
            nc.sync.dma_start(
                labf[:, i * LG : (i + 1) * LG],
                lab_ext[i * 128 : (i + 1) * 128, :],
            )

        slabs = []

        def emit_pred_dma(g):
            st = slab_pool.tile([128, FG // QB * NSTAT], fp16, tag="slab")
            nc.sync.dma_start(st, pred_ext[g * 128 : (g + 1) * 128, :])
            slabs.append(st)

        next_dma = 0
        while next_dma < 2 * (FC // FG):  # chunk-0 + chunk-1 groups
            emit_pred_dma(next_dma)
            next_dma += 1

        # ---- PE warmup on memset data during the DMA fill.
        nc.gpsimd.memset(warm_t[:], 1.0)
        nc.vector.memset(outt[:], 0.0)
        for w in range(WARM_MMS):
            nc.tensor.matmul(
                warm_ps[:64, :],
                warm_t[:, :64],
                warm_t[:, :128],
                start=(w == 0),
                stop=(w == WARM_MMS - 1),
            )

        # ---- main loop over one-hot chunks -------------------------------
        u = 0
        for ci in range(NCHUNKS):
            coff = ci * FC
            # pred DMA lookahead: everything the NEXT chunk needs.
            while next_dma < NGROUPS and next_dma * FG < coff + 2 * FC:
                emit_pred_dma(next_dma)
                next_dma += 1

            oh = oh_pool.tile([128, K * FC], fp16, tag="oh")
            oh2 = oh.rearrange("p (j t) -> p j t", j=K)
            lab_sl = labf[:, coff : coff + FC]
            for j in range(DVE_K):
                nc.vector.tensor_scalar(
                    oh2[:, j, :],
                    lab_sl,
                    float(j + 1),
                    None,
                    mybir.AluOpType.is_equal,
                )
            for j in range(DVE_K, DVE_K + GPS_K):
                nc.gpsimd.tensor_scalar(
                    oh2[:, j, :],
                    lab_sl,
                    float(j + 1),
                    None,
                    mybir.AluOpType.is_equal,
                )
            for j in range(DVE_K + GPS_K, K):
                at = tmp_pool.tile([128, FC], fp16, tag="atmp")
                nc.scalar.activation(
                    at[:],
                    lab_sl,
                    mybir.ActivationFunctionType.Square,
                    bias=bias_t[:, j - DVE_K - GPS_K : j - DVE_K - GPS_K + 1],
                )
                nc.scalar.activation(
                    oh2[:, j, :],
                    at[:],
                    mybir.ActivationFunctionType.Relu,
                    scale=-1.0,
                    bias=bias_t[:, ACT_K : ACT_K + 1],
                )

            for uc in range(FC // QB):
                g = u // (FG // QB)
                ug = u % (FG // QB)
                t = u % 2
                nc.tensor.matmul(
                    psum_s[64 * t : 64 * t + NSTAT, :],
                    slabs[g][:, ug * NSTAT : (ug + 1) * NSTAT],
                    oh2[:, :, uc * QB : (uc + 1) * QB],
                    start=(u < 2),
                    stop=(u >= NUNITS - 2),
                    tile_position=(0, 64 * t),
                    skip_group_check=True,
                )
                u += 1

        # ---- output ------------------------------------------------------
        nc.vector.tensor_copy(outt[:NSTAT, :NMOV], psum_s[:NSTAT, :])
        nc.vector.tensor_copy(
            outt[64 : 64 + NSTAT, :NMOV], psum_s[64 : 64 + NSTAT, :]
        )
        # keep the warm matmuls live (scratch cols, 32-aligned psum base)
        nc.vector.tensor_copy(outt[32:33, NMOV:], warm_ps[32:33, :8])
        nc.sync.dma_start(out_ext[:], outt[:])
    nc.compile()
    return nc


@functools.lru_cache(maxsize=1)
def _get_program():
    return build_nc()


def pack_core(pred_core, labels_core):
    """Host-side packing into the kernel's DMA layouts.

    pred -> fp16 * 2^14 in group-major stationary slabs
    [g, p, (u, b, c)] with the ones column baked in at c == 8;
    labels -> fp16 in group-major [gl, p, t] blocks.
    Pixel (p, t) = core_linear[p * FTOT + t].
    """
    ph = (
        np.asarray(pred_core, dtype=np.float32).reshape(C, 128, FTOT)
        * np.float32(PRED_SCALE)
    ).astype(np.float16)
    arr = np.empty((128, FTOT, NCH), dtype=np.float16)
    arr[:, :, :C] = ph.transpose(1, 2, 0)
    arr[:, :, C] = np.float16(1.0)
    pred_r = np.ascontiguousarray(
        arr.reshape(128, NGROUPS, FG * NCH).transpose(1, 0, 2)
    ).reshape(NGROUPS * 128, FG // QB * NSTAT)
    import ml_dtypes

    labf = labels_core.reshape(128, FTOT).astype(ml_dtypes.bfloat16)
    lab_r = np.ascontiguousarray(
        labf.reshape(128, FTOT // LG, LG).transpose(1, 0, 2)
    ).reshape((FTOT // LG) * 128, LG)
    return pred_r, lab_r


def make_in_maps(pred_flat, labels_flat, pcore=PCORE, ncores=NCORES):
    in_maps = []
    for i in range(ncores):
        sl = slice(i * pcore, (i + 1) * pcore)
        pred_r, lab_r = pack_core(pred_flat[:, sl], labels_flat[sl])
        in_maps.append({"pred": pred_r, "labels": lab_r})
    return in_maps


def extract_SN(res_core):
    """From one core's outputs: S_scaled [C, K] and N [K]."""
    ps = res_core["out_s"].astype(np.float64)[:, :NMOV]  # [128, NMOV]
    S = np.zeros((C, K))
    N = np.zeros(K)
    for t in range(2):
        r = ps[64 * t : 64 * t + NSTAT, :].reshape(QB, NCH, K, QB)
        d = r[np.arange(QB), :, :, np.arange(QB)].sum(axis=0)  # [NCH, K]
        S += d[:C, :]
        N += d[C, :]
    return S, N


def finish_host(results, num_kernel):
    S = np.zeros((C, K))
    N = np.zeros(K)
    for r in results:
        Si, Ni = extract_SN(r)
        S += Si
        N += Ni
    S /= PRED_SCALE
    A = N * np.sum(S * S, axis=0)  # [K]
    kk = int(num_kernel)
    A = A[:kk]
    pair = A[:, None] + A[None, :]
    Dm = np.maximum(SIGMA_DIS - np.sqrt(pair), 0.0)
    term = np.log(Dm * Dm + 1.0)
    L = float(np.sum(np.triu(term, k=1)))
    L *= (kk - 1) / kk
    return np.float32(L)


_last_results = None


def kernel(pred_similarities, regions_mask, kernel_labels, num_kernel, **kw):
    global _last_results
    from concourse.bass_utils import run_bass_kernel_spmd

    pred_flat = np.asarray(pred_similarities, dtype=np.float32).reshape(C, PTOT)
    labels_flat = np.asarray(kernel_labels, dtype=np.int32).reshape(PTOT)

    nc = _get_program()
    in_maps = make_in_maps(pred_flat, labels_flat)
    res = run_bass_kernel_spmd(nc, in_maps, list(range(NCORES)))
    _last_results = res
    return finish_host(
        [res.results[i] for i in range(NCORES)], num_kernel
    )
